# revision 31
# baseline (speedup 1.0000x reference)
"""Banded multi-headed attention (nn_BandedMultiheadedAttention) on 8 Trainium2 NeuronCores.

Sharding: data-parallel over (batch, sequence-chunk): core c handles batch c//4,
query positions [256*(c%4), 256*(c%4)+256). Band halo (max (KC-1)*dil = 248) is
loaded per-core with zero padding (projection of zero rows reproduces the
reference's bias padding exactly).

All-fp16 pipeline (fp32 PSUM accumulation everywhere, fp32 softmax). All DRAM
input layouts are host-packed so every SBUF partition's data is one contiguous
run (DMA is descriptor-rate-bound otherwise).

  1. Q/K projections (fp16): qT_s/kT_s [dh=128, pos].
  2. Banded scores per subhead: dense scores [q, m] -> deinterleaved fp16 DRAM
     plane (per (c, s) slice), shear-gather back as band [q, 32], PE-transpose
     to bandT [32, q].
  3. Pos_Sampling (Sk) matmul + Sb -> score2 [q, (head, 32)] fp32, exp, per-head
     row-sum, reciprocal, normalize -> W [q, (head, 32)] fp16.
  4. W scatter -> zeroed DRAM plane per head in [q, m] layout (contiguous runs),
     bulk readback per q-chunk, PE-transpose 128-col chunks to W^T [m, q];
     (chunk, q-half) combinations that cannot hold band data are skipped via
     partial-width PSUM accumulation in PV.
  5. V projection (fp16, heads packed in N) -> v_h [m, dh] tiles, interleaved
     with the band/softmax phase to keep PE busy during DRAM round trips.
  6. PV: attnT_h [dh, q] = v_h^T @ W^T + Vb (softmax rows sum to 1).
  7. Collapse: out [q, 640] = sum_h attnT_h^T @ CkT_h + Cb; half 0 interleaved
     with PV per head, half 1 as tail.
"""

import contextlib
import ctypes
import sys
import types

import numpy as np

# ---------------------------------------------------------------- constants
B, N, D = 2, 1024, 640
DH, KC, SUBHEADS, HEADS = 128, 32, 5, 14
Q = 256                      # query positions per core
NCORES = 8
HALO = 124                   # (KC-1)*max_dil // 2
KV = 512                     # kv halo positions per core ([t0-124, t0+388))
KVX = 1024                   # zero-extended vT columns

DIL_S = [1, 1, 2, 4, 8]
SUPER = [5, 5, 2, 1, 1]
DIL_H = [1] * 10 + [2, 2, 4, 8]
PL_S = [(KC - 1) * d // 2 for d in DIL_S]          # [15,15,31,62,124]
OFF_S = [HALO - p for p in PL_S]                   # kT col of m=0 per subhead
PL_H = [(KC - 1) * d // 2 for d in DIL_H]
OFF_H = [HALO - p for p in PL_H]

M_S = [288, 288, 320, 384, 512]                    # scores plane width per subhead
SOFF = [0, 288, 576, 896, 1280]
SLD = 1792                                         # scores plane row stride

# W plane width per head: md = M_H/dil must be a multiple of 128 so that each
# 128-col chunk of the deinterleaved plane lies in a single residue class
# (matmul stationary APs must be 2D, so V tiles need single-stride rows).
M_H = [384] * 10 + [512, 512, 512, 1024]
WOFF = [0]
for _m in M_H[:-1]:
    WOFF.append(WOFF[-1] + _m)
WLD = WOFF[-1] + M_H[-1]                           # 5632
MC_H = [m // 128 for m in M_H]

# V-projection head packs (same dilation within a pack)
PACKS = [[0, 1, 2, 3], [4, 5, 6, 7], [8, 9], [10, 11], [12], [13]]
PACK_OF_H = {h: (p, g.index(h)) for p, g in enumerate(PACKS) for h in g}
PACK_OFF = [OFF_H[g[0]] for g in PACKS]
PACK_MC = [MC_H[g[0]] for g in PACKS]

HJ = HEADS * KC  # 448


def _chunk_rows(h_or_p, mc, head=True):
    """Row (t, rho) segments of 128-col W-plane chunk mc: returns list of
    (row_in_chunk, rho, t0, seg_len); positions are p = OFF + dil*t + rho."""
    dil = DIL_H[h_or_p] if head else DIL_H[PACKS[h_or_p][0]]
    M = M_H[h_or_p] if head else M_H[PACKS[h_or_p][0]]
    md = M // dil
    segs = []
    r = 0
    while r < 128:
        col0 = mc * 128 + r
        rho, t0 = col0 // md, col0 % md
        seg = min(128 - r, md - t0)
        segs.append((r, rho, t0, seg))
        r += seg
    return segs


def _live_cs(h, mc):
    """Which q-halves c have any band data in W-plane chunk (h, mc)."""
    dil = DIL_H[h]
    lives = []
    for c in range(2):
        lo, hi = c * 128, c * 128 + 127 + (KC - 1) * dil
        ok = False
        for _, rho, t0, seg in _chunk_rows(h, mc):
            p0, p1 = dil * t0 + rho, dil * (t0 + seg - 1) + rho
            if p0 <= hi and p1 >= lo:
                ok = True
        if ok:
            lives.append(c)
    return lives


_BUILT = None


def _inject_ntff_hook():
    """bass_utils reads antenv.axon_hooks for NTFF profiling; the module is
    absent in this image. Recreate the ctypes glue (mirrors trn_boot.py)."""
    try:
        import antenv.axon_hooks  # noqa: F401
        return
    except ImportError:
        pass

    def _make(so_path):
        try:
            lib = ctypes.CDLL(so_path)
        except OSError:
            return None
        if not hasattr(lib, "axon_start_nrt_profile"):
            return None
        lib.axon_start_nrt_profile.argtypes = [ctypes.POINTER(ctypes.c_int64), ctypes.c_size_t]
        lib.axon_start_nrt_profile.restype = ctypes.c_int64
        lib.axon_stop_nrt_profile.argtypes = [ctypes.c_char_p]
        lib.axon_stop_nrt_profile.restype = ctypes.c_int64

        @contextlib.contextmanager
        def _hook(output_dir, device_ids):
            import jax
            jax.devices()
            if device_ids:
                ids = (ctypes.c_int64 * len(device_ids))(*device_ids)
                rc = lib.axon_start_nrt_profile(ids, len(device_ids))
            else:
                rc = lib.axon_start_nrt_profile(None, 0)
            if rc != 0:
                raise RuntimeError(f"axon_start_nrt_profile rc={rc}")
            try:
                yield
            finally:
                n = lib.axon_stop_nrt_profile(str(output_dir).encode())
                print(f"ntff profile: {n} file(s) -> {output_dir}", file=sys.stderr)

        return _hook

    hook = _make("/opt/axon/libaxon_pjrt.so")
    mod = types.ModuleType("antenv.axon_hooks")
    mod.get_axon_ntff_profile_hook = lambda: hook
    mod.set_axon_ntff_profile_hook = lambda h: None
    sys.modules["antenv.axon_hooks"] = mod


def _build():
    """Build the (single) SPMD Bass program. Returns finalized nc."""
    import concourse.bass as bass
    import concourse.tile as tile
    from concourse import bacc, mybir
    from concourse.masks import make_identity
    from concourse.tile import add_dep_helper

    f32 = mybir.dt.float32
    f16 = mybir.dt.float16
    AP = bass.AP

    nc = bacc.Bacc("TRN2", target_bir_lowering=False, debug=False, num_devices=NCORES)

    # ---------------- external IO (all fp16 except fp32 biases / output)
    # every input is host-packed [128, free] partition-major
    qT_d = nc.dram_tensor("qT", [DH, SUBHEADS * Q], f16, kind="ExternalInput")
    kT_d = nc.dram_tensor("kT", [DH, SUBHEADS * KV], f16, kind="ExternalInput")
    vT_d = nc.dram_tensor("vT", [DH, SUBHEADS * KV], f16, kind="ExternalInput")
    QkT_d = nc.dram_tensor("QkT", [DH, SUBHEADS * SUBHEADS * DH], f16, kind="ExternalInput")
    KkT_d = nc.dram_tensor("KkT", [DH, SUBHEADS * SUBHEADS * DH], f16, kind="ExternalInput")
    # V weights in three pack groups: heads 0-3, 4-7, 8-13
    VG = [512, 512, 768]
    VGP = [[0], [1], [2, 3, 4, 5]]  # packs per group
    Vg_d = [nc.dram_tensor(f"Vg{i}", [DH, SUBHEADS * w], f16, kind="ExternalInput")
            for i, w in enumerate(VG)]
    SkT_d = nc.dram_tensor("SkT", [KC, HJ], f16, kind="ExternalInput")
    Sb_d = nc.dram_tensor("Sb", [1, HJ], f32, kind="ExternalInput")
    bias3_d = nc.dram_tensor("bias3", [DH, 2 * SUBHEADS + HEADS], f32,
                             kind="ExternalInput")
    CkT_d = nc.dram_tensor("CkT", [DH, HEADS * D], f16, kind="ExternalInput")
    Cb_d = nc.dram_tensor("Cb", [1, D], f32, kind="ExternalInput")
    out_d = nc.dram_tensor("out", [Q, D], f32, kind="ExternalOutput")

    # ---------------- internal DRAM scratch, split per q-chunk so the tile
    # framework's DRAM dependency tracking stays per-chunk.
    splane = [nc.dram_tensor(f"splane{c}", [128, SLD], f16, kind="Internal")
              for c in range(2)]
    wplane = [nc.dram_tensor(f"wplane{c}", [128, WLD], f16, kind="Internal")
              for c in range(2)]

    with tile.TileContext(nc) as tc, contextlib.ExitStack() as ctx:
        consts = ctx.enter_context(tc.tile_pool(name="consts", bufs=1))
        acts = ctx.enter_context(tc.tile_pool(name="acts", bufs=1))
        work = ctx.enter_context(tc.tile_pool(name="work", bufs=4))
        wftp = ctx.enter_context(tc.tile_pool(name="wft", bufs=6))
        actp = ctx.enter_context(tc.tile_pool(name="actp", bufs=2))
        ps_mm = ctx.enter_context(tc.tile_pool(name="ps_mm", bufs=2, space="PSUM"))
        ps_sm = ctx.enter_context(tc.tile_pool(name="ps_sm", bufs=2, space="PSUM"))
        ps_at = ctx.enter_context(tc.tile_pool(name="ps_at", bufs=2, space="PSUM"))
        ps_co = ctx.enter_context(tc.tile_pool(name="ps_co", bufs=2, space="PSUM"))

        eng2 = [nc.sync, nc.scalar]
        cpy2 = [nc.scalar, nc.vector]

        # ---------------- critical inputs first: qT + QkT (then kT + KkT),
        # halves split across the two HWDGE engines; big contiguous runs.
        qTr = acts.tile([DH, SUBHEADS, Q], f16)
        kTr = acts.tile([DH, SUBHEADS, KV], f16)
        QkTr = consts.tile([DH, SUBHEADS * SUBHEADS, DH], f16)
        KkTr = consts.tile([DH, SUBHEADS * SUBHEADS, DH], f16)

        def pieces(eng, dst, src_d, width, npc):
            ds = []
            for i in range(npc):
                a, b = width * i // npc, width * (i + 1) // npc
                ds.append(eng.dma_start(
                    out=dst[:, a:b], in_=AP(src_d, a, [[width, DH], [1, b - a]])))
            return ds

        qTrf = qTr.rearrange("p a b -> p (a b)")
        kTrf = kTr.rearrange("p a b -> p (a b)")
        QkTrf = QkTr.rearrange("p a b -> p (a b)")
        KkTrf = KkTr.rearrange("p a b -> p (a b)")
        g1 = pieces(nc.sync, qTrf, qT_d, SUBHEADS * Q, 2)
        g2 = pieces(nc.scalar, QkTrf, QkT_d, SUBHEADS * SUBHEADS * DH, 3)
        bias3 = consts.tile([DH, 2 * SUBHEADS + HEADS], f32)
        nc.sync.dma_start(out=bias3, in_=bias3_d.ap())
        g3 = pieces(nc.sync, kTrf, kT_d, SUBHEADS * KV, 2)
        g4 = pieces(nc.scalar, KkTrf, KkT_d, SUBHEADS * SUBHEADS * DH, 3)
        gates = [g1[-1], g2[-1], g3[-1], g4[-1]]
        QbT = bias3[:, 0:SUBHEADS]
        KbT = bias3[:, SUBHEADS : 2 * SUBHEADS]
        VbT = bias3[:, 2 * SUBHEADS :]

        # ---------------- bulk loads on SWDGE (Pool engine)
        def gated(d):
            for g in gates:
                add_dep_helper(d.ins, g.ins, sync=True,
                               reason="defer bulk DMA until critical inputs loaded")
            return d

        vT = acts.tile([DH, SUBHEADS, KVX], f16)
        nc.vector.memset(vT[:, :, KV:], 0.0)
        gated(nc.gpsimd.dma_start(
            out=AP(vT.tensor, vT.offset,
                   [[SUBHEADS * KVX, DH], [KVX, SUBHEADS], [1, KV]]),
            in_=AP(vT_d, 0, [[SUBHEADS * KV, DH], [KV, SUBHEADS], [1, KV]])))
        Vgt = [consts.tile([DH, SUBHEADS, w], f16, name=f"Vg{i}")
               for i, w in enumerate(VG)]
        for i in range(3):
            gated(nc.gpsimd.dma_start(out=Vgt[i].rearrange("p a b -> p (a b)"),
                                      in_=Vg_d[i].ap()))
        # per-pack views into the groups
        Vp = []
        for i, ps in enumerate(VGP):
            off = 0
            for p in ps:
                npk = len(PACKS[p]) * DH
                Vp.append(Vgt[i][:, :, off : off + npk])
                off += npk
        SkT = consts.tile([KC, HJ], f16)
        nc.gpsimd.dma_start(out=SkT, in_=SkT_d.ap())
        Sb = consts.tile([DH, HJ], f32)
        nc.gpsimd.dma_start(out=Sb, in_=AP(Sb_d, 0, [[0, DH], [1, HJ]]))

        # zero the W planes (one fat DMA per plane; small zero source repeated);
        # deferred further (below) so they don't steal bandwidth from the
        # score-plane round trip
        zrow = work.tile([DH, WLD // 5], f16, name="zrow", tag="zr", bufs=1)
        nc.vector.memset(zrow, 0.0)
        zero_dmas = []
        for c in range(2):
            zero_dmas.append(gated(nc.gpsimd.dma_start(
                out=AP(wplane[c], 0, [[WLD, 128], [1, WLD]]),
                in_=AP(zrow.tensor, zrow.offset,
                       [[WLD // 5, DH], [0, 5], [1, WLD // 5]]))))

        CkT = consts.tile([DH, HEADS, D], f16)   # f-chunk h on partitions' free dim
        gated(nc.gpsimd.dma_start(out=CkT.rearrange("p a b -> p (a b)"),
                                  in_=CkT_d.ap()))
        Cb = consts.tile([DH, D], f32)
        gated(nc.gpsimd.dma_start(out=Cb, in_=AP(Cb_d, 0, [[0, DH], [1, D]])))

        ident = consts.tile([DH, DH], f32)
        make_identity(nc, ident)
        identh = consts.tile([DH, DH], f16)
        nc.vector.tensor_copy(identh, ident)

        # ---------------- Q/K projections (fp16 operands, fp32 PSUM)
        qTs, kTs = [], []
        for s in range(SUBHEADS):
            pq = ps_mm.tile([DH, Q], f32, name=f"pq{s}", tag="mm")
            for dc in range(SUBHEADS):
                nc.tensor.matmul(pq, QkTr[:, s * SUBHEADS + dc, :], qTr[:, dc, :],
                                 start=(dc == 0), stop=(dc == SUBHEADS - 1))
            t = acts.tile([DH, Q], f16, name=f"qTs{s}")
            nc.scalar.activation(t, pq, mybir.ActivationFunctionType.Identity,
                                 bias=QbT[:, s : s + 1], scale=1.0)
            qTs.append(t)

            # K projection only over the kT window this subhead's scores read
            ms = M_S[s]
            pk = ps_mm.tile([DH, ms], f32, name=f"pk{s}", tag="mm")
            for dc in range(SUBHEADS):
                nc.tensor.matmul(pk,
                                 KkTr[:, s * SUBHEADS + dc, :],
                                 kTr[:, dc, OFF_S[s] : OFF_S[s] + ms],
                                 start=(dc == 0), stop=(dc == SUBHEADS - 1))
            t = acts.tile([DH, ms], f16, name=f"kTs{s}")
            nc.vector.tensor_add(
                t, pk, AP(bias3.tensor,
                          bias3.offset + SUBHEADS + s,
                          [[2 * SUBHEADS + HEADS, DH], [0, ms]]))
            kTs.append(t)

        # ---------------- banded scores -> deinterleaved DRAM planes
        # per (c, s) slice DMAs so the band gathers can start per subhead
        bands = {}  # (c, s) -> [128, KC] f16 view
        for c in range(2):
            ssb = work.tile([128, SLD], f16, name=f"ssb{c}", tag="ssb", bufs=2)
            for s in range(SUBHEADS):
                dil, ms = DIL_S[s], M_S[s]
                pscore = ps_mm.tile([128, ms], f32, name=f"psc{s}{c}", tag="mm")
                nc.tensor.matmul(pscore, qTs[s][:, c * 128 : c * 128 + 128],
                                 kTs[s], start=True, stop=True)
                if dil == 1:
                    psrc = pscore
                    dst = ssb[:, SOFF[s] : SOFF[s] + ms]
                else:
                    # deinterleave m -> (m%dil, m//dil) during PSUM->SBUF copy
                    psrc = AP(pscore.tensor, pscore.offset,
                              [[ms, 128], [1, dil], [dil, ms // dil]])
                    dst = AP(ssb.tensor, ssb.offset + SOFF[s],
                             [[SLD, 128], [ms // dil, dil], [1, ms // dil]])
                if c == 0:
                    nc.vector.tensor_copy(dst, psrc)
                else:
                    nc.scalar.copy(dst, psrc)
            _w = nc.sync.dma_start(
                out=AP(splane[c], 0, [[SLD, 128], [1, SLD]]), in_=ssb)
            if c == 0:
                for _z in zero_dmas:
                    add_dep_helper(_z.ins, _w.ins, sync=True,
                                   reason="zero W planes after score-plane write")
            band01 = work.tile([128, 2, KC], f16, name=f"band01_{c}", tag="band01",
                               bufs=2)
            nc.sync.dma_start(
                out=band01,
                in_=AP(splane[c], c * 128, [[SLD + 1, 128], [SOFF[1], 2], [1, KC]]))
            bands[(c, 0)] = band01[:, 0, :]
            bands[(c, 1)] = band01[:, 1, :]
            for s in range(2, SUBHEADS):
                dil, ms = DIL_S[s], M_S[s]
                band = work.tile([128, KC], f16, name=f"band{c}{s}", tag="band",
                                 bufs=6)
                nc.sync.dma_start(
                    out=band,
                    in_=AP(splane[c], SOFF[s] + (c * 128) // dil,
                           [[SLD + ms // dil, dil], [dil * SLD + 1, 128 // dil],
                            [1, KC]]))
                bands[(c, s)] = band

        # ---------------- V projection tiles (fp16), interleaved with the
        # band->Sk->softmax phase so the PE stays busy during DRAM round trips.
        vtiles = {}  # (pack, mc) -> [128, len(g)*128] f16; rows in deint m-order

        def vproj_packs(plist):
            for p in plist:
                g = PACKS[p]
                npk = len(g) * DH
                dil = DIL_H[g[0]]
                for mc in range(PACK_MC[p]):
                    pv = ps_mm.tile([128, npk], f32, name=f"pv{p}{mc}", tag="mm")
                    segs = _chunk_rows(p, mc, head=False)
                    for dc in range(SUBHEADS):
                        base = vT.offset + dc * KVX
                        if len(segs) == 1:
                            _, rho, t0, _ = segs[0]
                            lhsT = AP(vT.tensor, base + PACK_OFF[p] + dil * t0 + rho,
                                      [[SUBHEADS * KVX, DH], [dil, 128]])
                        else:
                            # md=64: two residues per chunk; rows = (rho, rho+1) x t
                            _, rho, t0, seg = segs[0]
                            assert seg == 64 and t0 == 0
                            lhsT = AP(vT.tensor, base + PACK_OFF[p] + rho,
                                      [[SUBHEADS * KVX, DH], [1, 2], [dil, 64]])
                        nc.tensor.matmul(pv, lhsT, Vp[p][:, dc, :],
                                         start=(dc == 0), stop=(dc == SUBHEADS - 1))
                    t = acts.tile([128, npk], f16, name=f"v{p}_{mc}")
                    if (p + mc) % 2 == 0:
                        nc.vector.tensor_copy(t, pv)
                    else:
                        nc.scalar.copy(t, pv)
                    vtiles[(p, mc)] = t

        # ---------------- band -> Sk -> softmax -> scatter, per chunk
        def band_phase(c):
            bandTs = []
            for s in range(SUBHEADS):
                pbt = ps_sm.tile([KC, 128], f16, name="pbt", tag="sm")
                nc.tensor.transpose(pbt, bands[(c, s)], identh)
                bt = work.tile([KC, 128], f16, name="bt", tag="bt", bufs=6)
                nc.scalar.copy(bt, pbt)
                bandTs.append(bt)

            e = work.tile([128, HJ], f32, name="e", tag="e", bufs=2)
            hlo = 0
            for s in range(SUBHEADS):
                ncols = SUPER[s] * KC
                psk = ps_sm.tile([128, ncols], f32, name="psk", tag="sm")
                nc.tensor.matmul(psk, bandTs[s], SkT[:, hlo : hlo + ncols],
                                 start=True, stop=True)
                nc.vector.tensor_add(e[:, hlo : hlo + ncols], psk,
                                     Sb[:, hlo : hlo + ncols])
                hlo += ncols
            nc.scalar.activation(e, e, mybir.ActivationFunctionType.Exp)
            z = work.tile([128, HEADS], f32, name="z", tag="z", bufs=4)
            nc.vector.reduce_sum(z, e.rearrange("p (h k) -> p h k", k=KC),
                                 axis=mybir.AxisListType.X)
            rz = work.tile([128, HEADS], f32, name="rz", tag="z", bufs=4)
            nc.vector.reciprocal(rz, z)
            w = work.tile([128, HJ], f16, name="w", tag="w", bufs=2)
            nc.vector.tensor_mul(
                w.rearrange("p (h k) -> p h k", k=KC),
                e.rearrange("p (h k) -> p h k", k=KC),
                AP(rz.tensor, rz.offset, [[HEADS, 128], [1, HEADS], [0, KC]]),
            )

            # scatter W into the zeroed plane ([q, m] layout, contiguous runs)
            # dil=1 heads 0..9 merged into one DMA
            nc.sync.dma_start(
                out=AP(wplane[c], c * 128, [[WLD + 1, 128], [384, 10], [1, KC]]),
                in_=AP(w.tensor, w.offset, [[HJ, 128], [KC, 10], [1, KC]]),
            )
            for h in range(10, HEADS):
                dil, mh = DIL_H[h], M_H[h]
                base = WOFF[h] + (c * 128) // dil
                nc.sync.dma_start(
                    out=AP(wplane[c], base,
                           [[WLD + mh // dil, dil], [dil * WLD + 1, 128 // dil],
                            [1, KC]]),
                    in_=AP(w.tensor, w.offset + h * KC, [[HJ, 128], [1, KC]]),
                )
            # bulk readback of this chunk's W rows (waits on the scatters),
            # split in head-consumption order so the first W^T transposes
            # start after the first piece lands
            t = acts.tile([128, WLD], f16, name=f"wpl{c}")
            for i in range(3):
                a, b = WLD * i // 3, WLD * (i + 1) // 3
                nc.sync.dma_start(
                    out=t[:, a:b], in_=AP(wplane[c], a, [[WLD, 128], [1, b - a]]))
            return t

        # interleave: bands c0, Vproj pack1, bands c1, Vproj packs 2-5
        wpl = [None, None]
        vproj_packs([0])
        wpl[0] = band_phase(0)
        vproj_packs([1])
        wpl[1] = band_phase(1)
        vproj_packs([2, 3, 4, 5])

        # ---------------- W^T via PE transposes + PV + collapse
        # (collapse half 0 interleaved one head behind PV; half 1 as tail)
        atiles = []
        pcs = {}

        def collapse_half0(h):
            for cc in range(2):
                if h == 0:
                    pcs[cc] = ps_co.tile([128, 320], f32, name=f"pc{cc}", tag="co")
                nc.tensor.matmul(pcs[cc], atiles[h][:, cc * 128 : cc * 128 + 128],
                                 CkT[:, h, 0:320],
                                 start=(h == 0), stop=(h == HEADS - 1),
                                 skip_group_check=True)

        cpy3 = [nc.vector, nc.scalar]
        ncp = 0
        for h in range(HEADS):
            p, hh = PACK_OF_H[h]
            pat = ps_at.tile([DH, Q], f32, name=f"pat{h}", tag="at")
            groups = [(mc, _live_cs(h, mc)) for mc in range(MC_H[h])]
            # batches of <=4 live (mc, c) chunks, never splitting an mc pair;
            # 4 PE transposes feed ONE PSUM->SBUF copy (copies are
            # fixed-overhead dominated)
            batches, cur = [], []
            for mc, lv in groups:
                if len(cur) + len(lv) > 4:
                    batches.append(cur)
                    cur = []
                cur.extend((mc, c) for c in lv)
            if cur:
                batches.append(cur)
            pos = {}
            for batch in batches:
                nb = len(batch)
                ptp = ps_sm.tile([128, nb * 128], f16, name="ptp", tag="sm")
                for i, (mc, c) in enumerate(batch):
                    nc.tensor.transpose(
                        ptp[:, i * 128 : i * 128 + 128],
                        wpl[c][:, WOFF[h] + mc * 128 : WOFF[h] + mc * 128 + 128],
                        identh)
                wt = wftp.tile([128, nb * 128], f16, name="wft", tag="wft")
                eng = cpy3[ncp % 2]
                if eng is nc.scalar:
                    eng.copy(wt, ptp)
                else:
                    eng.tensor_copy(wt, ptp)
                ncp += 1
                for i, (mc, c) in enumerate(batch):
                    pos[(mc, c)] = (wt, i)
            fulls = [mc for mc, lv in groups if len(lv) == 2]
            parts = [(mc, lv[0]) for mc, lv in groups if len(lv) == 1]
            for i, mc in enumerate(fulls):
                wt, j = pos[(mc, 0)]
                nc.tensor.matmul(pat, vtiles[(p, mc)][:, hh * DH : hh * DH + DH],
                                 wt[:, j * 128 : j * 128 + 256],
                                 start=(i == 0),
                                 stop=(not parts and i == len(fulls) - 1),
                                 skip_group_check=True)
            for j2, (mc, c) in enumerate(parts):
                wt, j = pos[(mc, c)]
                nc.tensor.matmul(pat[:, c * 128 : c * 128 + 128],
                                 vtiles[(p, mc)][:, hh * DH : hh * DH + DH],
                                 wt[:, j * 128 : j * 128 + 128],
                                 start=False, stop=(j2 == len(parts) - 1),
                                 skip_group_check=True)
            at = actp.tile([DH, Q], f16, name=f"at{h}", tag="at", bufs=14)
            nc.vector.tensor_add(
                at, pat,
                AP(bias3.tensor, bias3.offset + 2 * SUBHEADS + h,
                   [[2 * SUBHEADS + HEADS, DH], [0, Q]]))
            atiles.append(at)
            if h > 0:
                collapse_half0(h - 1)
        collapse_half0(HEADS - 1)

        outsb = [work.tile([128, D], f32, name=f"osb{c}", tag="osb", bufs=2)
                 for c in range(2)]
        def out_dma(cc, half):
            for i in range(2):
                a = half * 320 + i * 160
                eng2[(cc + i) % 2].dma_start(
                    out=AP(out_d, cc * 128 * D + a, [[D, 128], [1, 160]]),
                    in_=outsb[cc][:, a : a + 160])

        for cc in range(2):
            nc.vector.tensor_add(outsb[cc][:, 0:320], pcs[cc], Cb[:, 0:320])
            out_dma(cc, 0)
        for cc in range(2):
            pc = ps_co.tile([128, 320], f32, name=f"pc2{cc}", tag="co")
            for h in range(HEADS):
                nc.tensor.matmul(pc, atiles[h][:, cc * 128 : cc * 128 + 128],
                                 CkT[:, h, 320:640],
                                 start=(h == 0), stop=(h == HEADS - 1))
            nc.vector.tensor_add(outsb[cc][:, 320:640], pc, Cb[:, 320:640])
            out_dma(cc, 1)

    nc.finalize()
    return nc


def _pack_rows(x, nchunk):
    """[nchunk*128, F] -> [128, nchunk*F] partition-major contiguous."""
    F = x.shape[1]
    return np.ascontiguousarray(
        x.reshape(nchunk, DH, F).transpose(1, 0, 2).reshape(DH, nchunk * F))


def _prep_in_maps(inputs):
    h16 = np.float16
    query = np.asarray(inputs["query"], np.float32)
    key = np.asarray(inputs["key"], np.float32)
    value = np.asarray(inputs["value"], np.float32)
    Qk = np.asarray(inputs["Qk"], np.float32)
    Qb = np.asarray(inputs["Qb"], np.float32)
    Kk = np.asarray(inputs["Kk"], np.float32)
    Kb = np.asarray(inputs["Kb"], np.float32)
    Vk = np.asarray(inputs["Vk"], np.float32)
    Vb = np.asarray(inputs["Vb"], np.float32)
    Sk = np.asarray(inputs["Sk"], np.float32)
    Sb = np.asarray(inputs["Sb"], np.float32)
    Ck = np.asarray(inputs["Ck"], np.float32)
    Cb = np.asarray(inputs["Cb"], np.float32)

    # QkT packed: [128, (s*5+dc)*128] with [d2, ., o] = Qk[s, o, dc*128+d2]
    def pack_w(Wk):  # [5, 128, 640] -> [128 d2, 25*128]
        WkT = Wk.transpose(0, 2, 1).reshape(SUBHEADS, SUBHEADS, DH, DH)  # s,dc,d2,o
        return np.ascontiguousarray(
            WkT.transpose(2, 0, 1, 3).reshape(DH, SUBHEADS * SUBHEADS * DH)).astype(h16)

    QkTp = pack_w(Qk)
    KkTp = pack_w(Kk)
    VkT = Vk.transpose(0, 2, 1)                                    # [14, 640, 128]
    VGH = [[0, 1, 2, 3], [4, 5, 6, 7], [8, 9, 10, 11, 12, 13]]     # heads per group
    Vgp = [_pack_rows(np.concatenate([VkT[h] for h in g], axis=1), SUBHEADS).astype(h16)
           for g in VGH]
    SkT = np.ascontiguousarray(Sk.transpose(2, 0, 1).reshape(KC, HJ)).astype(h16)
    Sbr = np.ascontiguousarray(Sb.reshape(1, HJ))
    bias3 = np.ascontiguousarray(
        np.concatenate([Qb.T, Kb.T, Vb.T], axis=1))                # [128, 24]
    CkTp = _pack_rows(np.ascontiguousarray(Ck.T), HEADS).astype(h16)  # [128, 14*640]
    Cbr = np.ascontiguousarray(Cb.reshape(1, D))

    in_maps = []
    for c in range(NCORES):
        b, t0 = c // 4, (c % 4) * Q
        kpad = np.zeros((KV, D), np.float32)
        vpad = np.zeros((KV, D), np.float32)
        lo, hi = max(0, t0 - HALO), min(N, t0 + Q + 132)
        kpad[lo - (t0 - HALO) : hi - (t0 - HALO)] = key[b, lo:hi]
        vpad[lo - (t0 - HALO) : hi - (t0 - HALO)] = value[b, lo:hi]
        m = {
            "qT": _pack_rows(query[b, t0 : t0 + Q].T, SUBHEADS).astype(h16),
            "kT": _pack_rows(kpad.T, SUBHEADS).astype(h16),
            "vT": _pack_rows(vpad.T, SUBHEADS).astype(h16),
            "QkT": QkTp, "KkT": KkTp,
            "SkT": SkT, "Sb": Sbr, "bias3": bias3,
            "CkT": CkTp, "Cb": Cbr,
        }
        for i in range(3):
            m[f"Vg{i}"] = Vgp[i]
        in_maps.append(m)
    return in_maps


def _run(inputs, trace=False, tmpdir=None):
    global _BUILT
    _inject_ntff_hook()
    from concourse.bass_utils import run_bass_kernel_spmd

    if _BUILT is None:
        _BUILT = _build()
    in_maps = _prep_in_maps(inputs)
    r = run_bass_kernel_spmd(_BUILT, in_maps, core_ids=list(range(NCORES)),
                             trace=trace, tmpdir=tmpdir)
    out = np.empty((B, N, D), np.float32)
    for c in range(NCORES):
        b, t0 = c // 4, (c % 4) * Q
        out[b, t0 : t0 + Q] = r.results[c]["out"]
    return out, r


def kernel(**inputs) -> np.ndarray:
    out, _ = _run(inputs, trace=False)
    return out


# revision 32
# speedup vs baseline: 1.0473x; 1.0473x over previous
"""Banded multi-headed attention (nn_BandedMultiheadedAttention) on 8 Trainium2 NeuronCores.

Sharding: data-parallel over (batch, sequence-chunk): core c handles batch c//4,
query positions [256*(c%4), 256*(c%4)+256). Band halo (max (KC-1)*dil = 248) is
loaded per-core with zero padding (projection of zero rows reproduces the
reference's bias padding exactly).

All-fp16 pipeline (fp32 PSUM accumulation everywhere, fp32 softmax). All DRAM
input layouts are host-packed so every SBUF partition's data is one contiguous
run (DMA is descriptor-rate-bound otherwise).

  1. Q/K projections (fp16): qT_s/kT_s [dh=128, pos].
  2. Banded scores per subhead: dense scores [q, m] -> deinterleaved fp16 DRAM
     plane (per (c, s) slice), shear-gather back as band [q, 32], PE-transpose
     to bandT [32, q].
  3. Pos_Sampling (Sk) matmul + Sb -> score2 [q, (head, 32)] fp32, exp, per-head
     row-sum, reciprocal, normalize -> W [q, (head, 32)] fp16.
  4. W scatter -> zeroed DRAM plane per head in [q, m] layout (contiguous runs),
     bulk readback per q-chunk, PE-transpose 128-col chunks to W^T [m, q];
     (chunk, q-half) combinations that cannot hold band data are skipped via
     partial-width PSUM accumulation in PV.
  5. V projection (fp16, heads packed in N) -> v_h [m, dh] tiles, interleaved
     with the band/softmax phase to keep PE busy during DRAM round trips.
  6. PV: attnT_h [dh, q] = v_h^T @ W^T + Vb (softmax rows sum to 1).
  7. Collapse: out [q, 640] = sum_h attnT_h^T @ CkT_h + Cb; half 0 interleaved
     with PV per head, half 1 as tail.
"""

import contextlib
import ctypes
import sys
import types

import numpy as np

# ---------------------------------------------------------------- constants
B, N, D = 2, 1024, 640
DH, KC, SUBHEADS, HEADS = 128, 32, 5, 14
Q = 256                      # query positions per core
NCORES = 8
HALO = 124                   # (KC-1)*max_dil // 2
KV = 512                     # kv halo positions per core ([t0-124, t0+388))
KVX = 1024                   # zero-extended vT columns

DIL_S = [1, 1, 2, 4, 8]
SUPER = [5, 5, 2, 1, 1]
DIL_H = [1] * 10 + [2, 2, 4, 8]
PL_S = [(KC - 1) * d // 2 for d in DIL_S]          # [15,15,31,62,124]
OFF_S = [HALO - p for p in PL_S]                   # kT col of m=0 per subhead
PL_H = [(KC - 1) * d // 2 for d in DIL_H]
OFF_H = [HALO - p for p in PL_H]

M_S = [288, 288, 320, 384, 512]                    # scores plane width per subhead
SOFF = [0, 288, 576, 896, 1280]
SLD = 1792                                         # scores plane row stride

# W plane width per head: md = M_H/dil must be a multiple of 128 so that each
# 128-col chunk of the deinterleaved plane lies in a single residue class
# (matmul stationary APs must be 2D, so V tiles need single-stride rows).
M_H = [384] * 10 + [512, 512, 512, 1024]
WOFF = [0]
for _m in M_H[:-1]:
    WOFF.append(WOFF[-1] + _m)
WLD = WOFF[-1] + M_H[-1]                           # 5632
MC_H = [m // 128 for m in M_H]

# V-projection head packs (same dilation within a pack)
PACKS = [[0, 1, 2, 3], [4, 5, 6, 7], [8, 9], [10, 11], [12], [13]]
PACK_OF_H = {h: (p, g.index(h)) for p, g in enumerate(PACKS) for h in g}
PACK_OFF = [OFF_H[g[0]] for g in PACKS]
PACK_MC = [MC_H[g[0]] for g in PACKS]

HJ = HEADS * KC  # 448


def _chunk_rows(h_or_p, mc, head=True):
    """Row (t, rho) segments of 128-col W-plane chunk mc: returns list of
    (row_in_chunk, rho, t0, seg_len); positions are p = OFF + dil*t + rho."""
    dil = DIL_H[h_or_p] if head else DIL_H[PACKS[h_or_p][0]]
    M = M_H[h_or_p] if head else M_H[PACKS[h_or_p][0]]
    md = M // dil
    segs = []
    r = 0
    while r < 128:
        col0 = mc * 128 + r
        rho, t0 = col0 // md, col0 % md
        seg = min(128 - r, md - t0)
        segs.append((r, rho, t0, seg))
        r += seg
    return segs


def _live_cs(h, mc):
    """Which q-halves c have any band data in W-plane chunk (h, mc)."""
    dil = DIL_H[h]
    lives = []
    for c in range(2):
        lo, hi = c * 128, c * 128 + 127 + (KC - 1) * dil
        ok = False
        for _, rho, t0, seg in _chunk_rows(h, mc):
            p0, p1 = dil * t0 + rho, dil * (t0 + seg - 1) + rho
            if p0 <= hi and p1 >= lo:
                ok = True
        if ok:
            lives.append(c)
    return lives


_BUILT = None


def _inject_ntff_hook():
    """bass_utils reads antenv.axon_hooks for NTFF profiling; the module is
    absent in this image. Recreate the ctypes glue (mirrors trn_boot.py)."""
    try:
        import antenv.axon_hooks  # noqa: F401
        return
    except ImportError:
        pass

    def _make(so_path):
        try:
            lib = ctypes.CDLL(so_path)
        except OSError:
            return None
        if not hasattr(lib, "axon_start_nrt_profile"):
            return None
        lib.axon_start_nrt_profile.argtypes = [ctypes.POINTER(ctypes.c_int64), ctypes.c_size_t]
        lib.axon_start_nrt_profile.restype = ctypes.c_int64
        lib.axon_stop_nrt_profile.argtypes = [ctypes.c_char_p]
        lib.axon_stop_nrt_profile.restype = ctypes.c_int64

        @contextlib.contextmanager
        def _hook(output_dir, device_ids):
            import jax
            jax.devices()
            if device_ids:
                ids = (ctypes.c_int64 * len(device_ids))(*device_ids)
                rc = lib.axon_start_nrt_profile(ids, len(device_ids))
            else:
                rc = lib.axon_start_nrt_profile(None, 0)
            if rc != 0:
                raise RuntimeError(f"axon_start_nrt_profile rc={rc}")
            try:
                yield
            finally:
                n = lib.axon_stop_nrt_profile(str(output_dir).encode())
                print(f"ntff profile: {n} file(s) -> {output_dir}", file=sys.stderr)

        return _hook

    hook = _make("/opt/axon/libaxon_pjrt.so")
    mod = types.ModuleType("antenv.axon_hooks")
    mod.get_axon_ntff_profile_hook = lambda: hook
    mod.set_axon_ntff_profile_hook = lambda h: None
    sys.modules["antenv.axon_hooks"] = mod


def _build():
    """Build the (single) SPMD Bass program. Returns finalized nc."""
    import concourse.bass as bass
    import concourse.tile as tile
    from concourse import bacc, mybir
    from concourse.masks import make_identity
    from concourse.tile import add_dep_helper

    f32 = mybir.dt.float32
    f16 = mybir.dt.float16
    AP = bass.AP

    nc = bacc.Bacc("TRN2", target_bir_lowering=False, debug=False, num_devices=NCORES)

    # ---------------- external IO (all fp16 except fp32 biases / output)
    # every input is host-packed [128, free] partition-major
    qT_d = nc.dram_tensor("qT", [DH, SUBHEADS * Q], f16, kind="ExternalInput")
    kT_d = nc.dram_tensor("kT", [DH, SUBHEADS * KV], f16, kind="ExternalInput")
    vT_d = nc.dram_tensor("vT", [DH, SUBHEADS * KV], f16, kind="ExternalInput")
    QkT_d = nc.dram_tensor("QkT", [DH, SUBHEADS * SUBHEADS * DH], f16, kind="ExternalInput")
    KkT_d = nc.dram_tensor("KkT", [DH, SUBHEADS * SUBHEADS * DH], f16, kind="ExternalInput")
    # V weights in three pack groups: heads 0-3, 4-7, 8-13
    VG = [512, 512, 768]
    VGP = [[0], [1], [2, 3, 4, 5]]  # packs per group
    Vg_d = [nc.dram_tensor(f"Vg{i}", [DH, SUBHEADS * w], f16, kind="ExternalInput")
            for i, w in enumerate(VG)]
    SkT_d = nc.dram_tensor("SkT", [KC, HJ], f16, kind="ExternalInput")
    Sb_d = nc.dram_tensor("Sb", [1, HJ], f32, kind="ExternalInput")
    bias3_d = nc.dram_tensor("bias3", [DH, 2 * SUBHEADS + HEADS], f32,
                             kind="ExternalInput")
    CkT_d = nc.dram_tensor("CkT", [DH, HEADS * D], f16, kind="ExternalInput")
    Cb_d = nc.dram_tensor("Cb", [1, D], f32, kind="ExternalInput")
    out_d = nc.dram_tensor("out", [Q, D], f32, kind="ExternalOutput")

    # ---------------- internal DRAM scratch, split per q-chunk so the tile
    # framework's DRAM dependency tracking stays per-chunk.
    splane = [nc.dram_tensor(f"splane{c}", [128, SLD], f16, kind="Internal")
              for c in range(2)]
    wplane = [nc.dram_tensor(f"wplane{c}", [128, WLD], f16, kind="Internal")
              for c in range(2)]

    with tile.TileContext(nc) as tc, contextlib.ExitStack() as ctx:
        consts = ctx.enter_context(tc.tile_pool(name="consts", bufs=1))
        acts = ctx.enter_context(tc.tile_pool(name="acts", bufs=1))
        work = ctx.enter_context(tc.tile_pool(name="work", bufs=4))
        wftp = ctx.enter_context(tc.tile_pool(name="wft", bufs=6))
        actp = ctx.enter_context(tc.tile_pool(name="actp", bufs=2))
        ps_mm = ctx.enter_context(tc.tile_pool(name="ps_mm", bufs=2, space="PSUM"))
        ps_sm = ctx.enter_context(tc.tile_pool(name="ps_sm", bufs=2, space="PSUM"))
        ps_at = ctx.enter_context(tc.tile_pool(name="ps_at", bufs=2, space="PSUM"))
        ps_co = ctx.enter_context(tc.tile_pool(name="ps_co", bufs=2, space="PSUM"))

        eng2 = [nc.sync, nc.scalar]
        cpy2 = [nc.scalar, nc.vector]

        # ---------------- critical inputs first: qT + QkT (then kT + KkT),
        # halves split across the two HWDGE engines; big contiguous runs.
        qTr = acts.tile([DH, SUBHEADS, Q], f16)
        kTr = acts.tile([DH, SUBHEADS, KV], f16)
        QkTr = consts.tile([DH, SUBHEADS * SUBHEADS, DH], f16)
        KkTr = consts.tile([DH, SUBHEADS * SUBHEADS, DH], f16)

        def pieces(eng, dst, src_d, width, npc):
            ds = []
            for i in range(npc):
                a, b = width * i // npc, width * (i + 1) // npc
                ds.append(eng.dma_start(
                    out=dst[:, a:b], in_=AP(src_d, a, [[width, DH], [1, b - a]])))
            return ds

        qTrf = qTr.rearrange("p a b -> p (a b)")
        kTrf = kTr.rearrange("p a b -> p (a b)")
        QkTrf = QkTr.rearrange("p a b -> p (a b)")
        KkTrf = KkTr.rearrange("p a b -> p (a b)")
        g1 = pieces(nc.sync, qTrf, qT_d, SUBHEADS * Q, 2)
        g2 = pieces(nc.scalar, QkTrf, QkT_d, SUBHEADS * SUBHEADS * DH, 3)
        bias3 = consts.tile([DH, 2 * SUBHEADS + HEADS], f32)
        nc.sync.dma_start(out=bias3, in_=bias3_d.ap())
        g3 = pieces(nc.sync, kTrf, kT_d, SUBHEADS * KV, 2)
        g4 = pieces(nc.scalar, KkTrf, KkT_d, SUBHEADS * SUBHEADS * DH, 3)
        gates = [g1[-1], g2[-1], g3[-1], g4[-1]]
        QbT = bias3[:, 0:SUBHEADS]
        KbT = bias3[:, SUBHEADS : 2 * SUBHEADS]
        VbT = bias3[:, 2 * SUBHEADS :]

        # ---------------- bulk loads on SWDGE (Pool engine)
        def gated(d):
            for g in gates:
                add_dep_helper(d.ins, g.ins, sync=True,
                               reason="defer bulk DMA until critical inputs loaded")
            return d

        vT = acts.tile([DH, SUBHEADS, KVX], f16)
        nc.vector.memset(vT[:, :, KV:], 0.0)
        gated(nc.gpsimd.dma_start(
            out=AP(vT.tensor, vT.offset,
                   [[SUBHEADS * KVX, DH], [KVX, SUBHEADS], [1, KV]]),
            in_=AP(vT_d, 0, [[SUBHEADS * KV, DH], [KV, SUBHEADS], [1, KV]])))
        Vgt = [consts.tile([DH, SUBHEADS, w], f16, name=f"Vg{i}")
               for i, w in enumerate(VG)]
        for i in range(3):
            gated(nc.gpsimd.dma_start(out=Vgt[i].rearrange("p a b -> p (a b)"),
                                      in_=Vg_d[i].ap()))
        # per-pack views into the groups
        Vp = []
        for i, ps in enumerate(VGP):
            off = 0
            for p in ps:
                npk = len(PACKS[p]) * DH
                Vp.append(Vgt[i][:, :, off : off + npk])
                off += npk
        SkT = consts.tile([KC, HJ], f16)
        nc.gpsimd.dma_start(out=SkT, in_=SkT_d.ap())
        Sb = consts.tile([DH, HJ], f32)
        nc.gpsimd.dma_start(out=Sb, in_=AP(Sb_d, 0, [[0, DH], [1, HJ]]))

        # zero the W planes (one fat DMA per plane; small zero source repeated)
        zrow = work.tile([DH, WLD // 5], f16, name="zrow", tag="zr", bufs=1)
        nc.vector.memset(zrow, 0.0)
        for c in range(2):
            gated(nc.gpsimd.dma_start(
                out=AP(wplane[c], 0, [[WLD, 128], [1, WLD]]),
                in_=AP(zrow.tensor, zrow.offset,
                       [[WLD // 5, DH], [0, 5], [1, WLD // 5]])))

        CkT = consts.tile([DH, HEADS, D], f16)   # f-chunk h on partitions' free dim
        gated(nc.gpsimd.dma_start(out=CkT.rearrange("p a b -> p (a b)"),
                                  in_=CkT_d.ap()))
        Cb = consts.tile([DH, D], f32)
        gated(nc.gpsimd.dma_start(out=Cb, in_=AP(Cb_d, 0, [[0, DH], [1, D]])))

        ident = consts.tile([DH, DH], f32)
        make_identity(nc, ident)
        identh = consts.tile([DH, DH], f16)
        nc.vector.tensor_copy(identh, ident)

        # ---------------- Q/K projections (fp16 operands, fp32 PSUM)
        qTs, kTs = [], []
        for s in range(SUBHEADS):
            pq = ps_mm.tile([DH, Q], f32, name=f"pq{s}", tag="mm")
            for dc in range(SUBHEADS):
                nc.tensor.matmul(pq, QkTr[:, s * SUBHEADS + dc, :], qTr[:, dc, :],
                                 start=(dc == 0), stop=(dc == SUBHEADS - 1))
            t = acts.tile([DH, Q], f16, name=f"qTs{s}")
            nc.scalar.activation(t, pq, mybir.ActivationFunctionType.Identity,
                                 bias=QbT[:, s : s + 1], scale=1.0)
            qTs.append(t)

            # K projection only over the kT window this subhead's scores read
            ms = M_S[s]
            pk = ps_mm.tile([DH, ms], f32, name=f"pk{s}", tag="mm")
            for dc in range(SUBHEADS):
                nc.tensor.matmul(pk,
                                 KkTr[:, s * SUBHEADS + dc, :],
                                 kTr[:, dc, OFF_S[s] : OFF_S[s] + ms],
                                 start=(dc == 0), stop=(dc == SUBHEADS - 1))
            t = acts.tile([DH, ms], f16, name=f"kTs{s}")
            nc.vector.tensor_add(
                t, pk, AP(bias3.tensor,
                          bias3.offset + SUBHEADS + s,
                          [[2 * SUBHEADS + HEADS, DH], [0, ms]]))
            kTs.append(t)

        # ---------------- banded scores -> deinterleaved DRAM planes
        # per (c, s) slice DMAs so the band gathers can start per subhead
        bands = {}  # (c, s) -> [128, KC] f16 view
        for c in range(2):
            ssb = work.tile([128, SLD], f16, name=f"ssb{c}", tag="ssb", bufs=2)
            for s in range(SUBHEADS):
                dil, ms = DIL_S[s], M_S[s]
                pscore = ps_mm.tile([128, ms], f32, name=f"psc{s}{c}", tag="mm")
                nc.tensor.matmul(pscore, qTs[s][:, c * 128 : c * 128 + 128],
                                 kTs[s], start=True, stop=True)
                if dil == 1:
                    psrc = pscore
                    dst = ssb[:, SOFF[s] : SOFF[s] + ms]
                else:
                    # deinterleave m -> (m%dil, m//dil) during PSUM->SBUF copy
                    psrc = AP(pscore.tensor, pscore.offset,
                              [[ms, 128], [1, dil], [dil, ms // dil]])
                    dst = AP(ssb.tensor, ssb.offset + SOFF[s],
                             [[SLD, 128], [ms // dil, dil], [1, ms // dil]])
                if c == 0:
                    nc.vector.tensor_copy(dst, psrc)
                else:
                    nc.scalar.copy(dst, psrc)
            nc.sync.dma_start(
                out=AP(splane[c], 0, [[SLD, 128], [1, SLD]]), in_=ssb)
            band01 = work.tile([128, 2, KC], f16, name=f"band01_{c}", tag="band01",
                               bufs=2)
            nc.sync.dma_start(
                out=band01,
                in_=AP(splane[c], c * 128, [[SLD + 1, 128], [SOFF[1], 2], [1, KC]]))
            bands[(c, 0)] = band01[:, 0, :]
            bands[(c, 1)] = band01[:, 1, :]
            for s in range(2, SUBHEADS):
                dil, ms = DIL_S[s], M_S[s]
                band = work.tile([128, KC], f16, name=f"band{c}{s}", tag="band",
                                 bufs=6)
                nc.sync.dma_start(
                    out=band,
                    in_=AP(splane[c], SOFF[s] + (c * 128) // dil,
                           [[SLD + ms // dil, dil], [dil * SLD + 1, 128 // dil],
                            [1, KC]]))
                bands[(c, s)] = band

        # ---------------- V projection tiles (fp16), interleaved with the
        # band->Sk->softmax phase so the PE stays busy during DRAM round trips.
        vtiles = {}  # (pack, mc) -> [128, len(g)*128] f16; rows in deint m-order

        def vproj_packs(plist):
            for p in plist:
                g = PACKS[p]
                npk = len(g) * DH
                dil = DIL_H[g[0]]
                for mc in range(PACK_MC[p]):
                    pv = ps_mm.tile([128, npk], f32, name=f"pv{p}{mc}", tag="mm")
                    segs = _chunk_rows(p, mc, head=False)
                    for dc in range(SUBHEADS):
                        base = vT.offset + dc * KVX
                        if len(segs) == 1:
                            _, rho, t0, _ = segs[0]
                            lhsT = AP(vT.tensor, base + PACK_OFF[p] + dil * t0 + rho,
                                      [[SUBHEADS * KVX, DH], [dil, 128]])
                        else:
                            # md=64: two residues per chunk; rows = (rho, rho+1) x t
                            _, rho, t0, seg = segs[0]
                            assert seg == 64 and t0 == 0
                            lhsT = AP(vT.tensor, base + PACK_OFF[p] + rho,
                                      [[SUBHEADS * KVX, DH], [1, 2], [dil, 64]])
                        nc.tensor.matmul(pv, lhsT, Vp[p][:, dc, :],
                                         start=(dc == 0), stop=(dc == SUBHEADS - 1))
                    t = acts.tile([128, npk], f16, name=f"v{p}_{mc}")
                    if (p + mc) % 2 == 0:
                        nc.vector.tensor_copy(t, pv)
                    else:
                        nc.scalar.copy(t, pv)
                    vtiles[(p, mc)] = t

        # ---------------- band -> Sk -> softmax -> scatter, per chunk
        def band_phase(c):
            bandTs = []
            for s in range(SUBHEADS):
                pbt = ps_sm.tile([KC, 128], f16, name="pbt", tag="sm")
                nc.tensor.transpose(pbt, bands[(c, s)], identh)
                bt = work.tile([KC, 128], f16, name="bt", tag="bt", bufs=6)
                nc.scalar.copy(bt, pbt)
                bandTs.append(bt)

            e = work.tile([128, HJ], f32, name="e", tag="e", bufs=2)
            hlo = 0
            for s in range(SUBHEADS):
                ncols = SUPER[s] * KC
                psk = ps_sm.tile([128, ncols], f32, name="psk", tag="sm")
                nc.tensor.matmul(psk, bandTs[s], SkT[:, hlo : hlo + ncols],
                                 start=True, stop=True)
                nc.vector.tensor_add(e[:, hlo : hlo + ncols], psk,
                                     Sb[:, hlo : hlo + ncols])
                hlo += ncols
            nc.scalar.activation(e, e, mybir.ActivationFunctionType.Exp)
            z = work.tile([128, HEADS], f32, name="z", tag="z", bufs=4)
            nc.vector.reduce_sum(z, e.rearrange("p (h k) -> p h k", k=KC),
                                 axis=mybir.AxisListType.X)
            rz = work.tile([128, HEADS], f32, name="rz", tag="z", bufs=4)
            nc.vector.reciprocal(rz, z)
            w = work.tile([128, HJ], f16, name="w", tag="w", bufs=2)
            nc.vector.tensor_mul(
                w.rearrange("p (h k) -> p h k", k=KC),
                e.rearrange("p (h k) -> p h k", k=KC),
                AP(rz.tensor, rz.offset, [[HEADS, 128], [1, HEADS], [0, KC]]),
            )

            # scatter W into the zeroed plane ([q, m] layout, contiguous runs)
            # dil=1 heads 0..9 merged into one DMA
            nc.sync.dma_start(
                out=AP(wplane[c], c * 128, [[WLD + 1, 128], [384, 10], [1, KC]]),
                in_=AP(w.tensor, w.offset, [[HJ, 128], [KC, 10], [1, KC]]),
            )
            for h in range(10, HEADS):
                dil, mh = DIL_H[h], M_H[h]
                base = WOFF[h] + (c * 128) // dil
                nc.sync.dma_start(
                    out=AP(wplane[c], base,
                           [[WLD + mh // dil, dil], [dil * WLD + 1, 128 // dil],
                            [1, KC]]),
                    in_=AP(w.tensor, w.offset + h * KC, [[HJ, 128], [1, KC]]),
                )
            # bulk readback of this chunk's W rows (waits on the scatters),
            # split in head-consumption order so the first W^T transposes
            # start after the first piece lands
            t = acts.tile([128, WLD], f16, name=f"wpl{c}")
            for i in range(2):
                a, b = WLD * i // 2, WLD * (i + 1) // 2
                nc.sync.dma_start(
                    out=t[:, a:b], in_=AP(wplane[c], a, [[WLD, 128], [1, b - a]]))
            return t

        # interleave: bands c0, Vproj pack1, bands c1, Vproj packs 2-5
        wpl = [None, None]
        vproj_packs([0])
        wpl[0] = band_phase(0)
        vproj_packs([1])
        wpl[1] = band_phase(1)
        vproj_packs([2, 3, 4, 5])

        # ---------------- W^T via PE transposes + PV + collapse
        # (collapse half 0 interleaved one head behind PV; half 1 as tail)
        atiles = []
        pcs = {}

        def collapse_half0(h):
            for cc in range(2):
                if h == 0:
                    pcs[cc] = ps_co.tile([128, 320], f32, name=f"pc{cc}", tag="co")
                nc.tensor.matmul(pcs[cc], atiles[h][:, cc * 128 : cc * 128 + 128],
                                 CkT[:, h, 0:320],
                                 start=(h == 0), stop=(h == HEADS - 1),
                                 skip_group_check=True)

        cpy3 = [nc.vector, nc.scalar]
        ncp = 0
        for h in range(HEADS):
            p, hh = PACK_OF_H[h]
            pat = ps_at.tile([DH, Q], f32, name=f"pat{h}", tag="at")
            groups = [(mc, _live_cs(h, mc)) for mc in range(MC_H[h])]
            # batches of <=4 live (mc, c) chunks, never splitting an mc pair;
            # 4 PE transposes feed ONE PSUM->SBUF copy (copies are
            # fixed-overhead dominated)
            batches, cur = [], []
            for mc, lv in groups:
                if len(cur) + len(lv) > 4:
                    batches.append(cur)
                    cur = []
                cur.extend((mc, c) for c in lv)
            if cur:
                batches.append(cur)
            pos = {}
            for batch in batches:
                nb = len(batch)
                ptp = ps_sm.tile([128, nb * 128], f16, name="ptp", tag="sm")
                for i, (mc, c) in enumerate(batch):
                    nc.tensor.transpose(
                        ptp[:, i * 128 : i * 128 + 128],
                        wpl[c][:, WOFF[h] + mc * 128 : WOFF[h] + mc * 128 + 128],
                        identh)
                wt = wftp.tile([128, nb * 128], f16, name="wft", tag="wft")
                eng = cpy3[ncp % 2]
                if eng is nc.scalar:
                    eng.copy(wt, ptp)
                else:
                    eng.tensor_copy(wt, ptp)
                ncp += 1
                for i, (mc, c) in enumerate(batch):
                    pos[(mc, c)] = (wt, i)
            fulls = [mc for mc, lv in groups if len(lv) == 2]
            parts = [(mc, lv[0]) for mc, lv in groups if len(lv) == 1]
            for i, mc in enumerate(fulls):
                wt, j = pos[(mc, 0)]
                nc.tensor.matmul(pat, vtiles[(p, mc)][:, hh * DH : hh * DH + DH],
                                 wt[:, j * 128 : j * 128 + 256],
                                 start=(i == 0),
                                 stop=(not parts and i == len(fulls) - 1),
                                 skip_group_check=True)
            for j2, (mc, c) in enumerate(parts):
                wt, j = pos[(mc, c)]
                nc.tensor.matmul(pat[:, c * 128 : c * 128 + 128],
                                 vtiles[(p, mc)][:, hh * DH : hh * DH + DH],
                                 wt[:, j * 128 : j * 128 + 128],
                                 start=False, stop=(j2 == len(parts) - 1),
                                 skip_group_check=True)
            at = actp.tile([DH, Q], f16, name=f"at{h}", tag="at", bufs=14)
            nc.vector.tensor_add(
                at, pat,
                AP(bias3.tensor, bias3.offset + 2 * SUBHEADS + h,
                   [[2 * SUBHEADS + HEADS, DH], [0, Q]]))
            atiles.append(at)
            if h > 0:
                collapse_half0(h - 1)
        collapse_half0(HEADS - 1)

        outsb = [work.tile([128, D], f32, name=f"osb{c}", tag="osb", bufs=2)
                 for c in range(2)]
        def out_dma(cc, half):
            for i in range(2):
                a = half * 320 + i * 160
                eng2[(cc + i) % 2].dma_start(
                    out=AP(out_d, cc * 128 * D + a, [[D, 128], [1, 160]]),
                    in_=outsb[cc][:, a : a + 160])

        for cc in range(2):
            nc.vector.tensor_add(outsb[cc][:, 0:320], pcs[cc], Cb[:, 0:320])
            out_dma(cc, 0)
        for cc in range(2):
            pc = ps_co.tile([128, 320], f32, name=f"pc2{cc}", tag="co")
            for h in range(HEADS):
                nc.tensor.matmul(pc, atiles[h][:, cc * 128 : cc * 128 + 128],
                                 CkT[:, h, 320:640],
                                 start=(h == 0), stop=(h == HEADS - 1))
            nc.vector.tensor_add(outsb[cc][:, 320:640], pc, Cb[:, 320:640])
            out_dma(cc, 1)

    nc.finalize()
    return nc


def _pack_rows(x, nchunk):
    """[nchunk*128, F] -> [128, nchunk*F] partition-major contiguous."""
    F = x.shape[1]
    return np.ascontiguousarray(
        x.reshape(nchunk, DH, F).transpose(1, 0, 2).reshape(DH, nchunk * F))


def _prep_in_maps(inputs):
    h16 = np.float16
    query = np.asarray(inputs["query"], np.float32)
    key = np.asarray(inputs["key"], np.float32)
    value = np.asarray(inputs["value"], np.float32)
    Qk = np.asarray(inputs["Qk"], np.float32)
    Qb = np.asarray(inputs["Qb"], np.float32)
    Kk = np.asarray(inputs["Kk"], np.float32)
    Kb = np.asarray(inputs["Kb"], np.float32)
    Vk = np.asarray(inputs["Vk"], np.float32)
    Vb = np.asarray(inputs["Vb"], np.float32)
    Sk = np.asarray(inputs["Sk"], np.float32)
    Sb = np.asarray(inputs["Sb"], np.float32)
    Ck = np.asarray(inputs["Ck"], np.float32)
    Cb = np.asarray(inputs["Cb"], np.float32)

    # QkT packed: [128, (s*5+dc)*128] with [d2, ., o] = Qk[s, o, dc*128+d2]
    def pack_w(Wk):  # [5, 128, 640] -> [128 d2, 25*128]
        WkT = Wk.transpose(0, 2, 1).reshape(SUBHEADS, SUBHEADS, DH, DH)  # s,dc,d2,o
        return np.ascontiguousarray(
            WkT.transpose(2, 0, 1, 3).reshape(DH, SUBHEADS * SUBHEADS * DH)).astype(h16)

    QkTp = pack_w(Qk)
    KkTp = pack_w(Kk)
    VkT = Vk.transpose(0, 2, 1)                                    # [14, 640, 128]
    VGH = [[0, 1, 2, 3], [4, 5, 6, 7], [8, 9, 10, 11, 12, 13]]     # heads per group
    Vgp = [_pack_rows(np.concatenate([VkT[h] for h in g], axis=1), SUBHEADS).astype(h16)
           for g in VGH]
    SkT = np.ascontiguousarray(Sk.transpose(2, 0, 1).reshape(KC, HJ)).astype(h16)
    Sbr = np.ascontiguousarray(Sb.reshape(1, HJ))
    bias3 = np.ascontiguousarray(
        np.concatenate([Qb.T, Kb.T, Vb.T], axis=1))                # [128, 24]
    CkTp = _pack_rows(np.ascontiguousarray(Ck.T), HEADS).astype(h16)  # [128, 14*640]
    Cbr = np.ascontiguousarray(Cb.reshape(1, D))

    in_maps = []
    for c in range(NCORES):
        b, t0 = c // 4, (c % 4) * Q
        kpad = np.zeros((KV, D), np.float32)
        vpad = np.zeros((KV, D), np.float32)
        lo, hi = max(0, t0 - HALO), min(N, t0 + Q + 132)
        kpad[lo - (t0 - HALO) : hi - (t0 - HALO)] = key[b, lo:hi]
        vpad[lo - (t0 - HALO) : hi - (t0 - HALO)] = value[b, lo:hi]
        m = {
            "qT": _pack_rows(query[b, t0 : t0 + Q].T, SUBHEADS).astype(h16),
            "kT": _pack_rows(kpad.T, SUBHEADS).astype(h16),
            "vT": _pack_rows(vpad.T, SUBHEADS).astype(h16),
            "QkT": QkTp, "KkT": KkTp,
            "SkT": SkT, "Sb": Sbr, "bias3": bias3,
            "CkT": CkTp, "Cb": Cbr,
        }
        for i in range(3):
            m[f"Vg{i}"] = Vgp[i]
        in_maps.append(m)
    return in_maps


def _run(inputs, trace=False, tmpdir=None):
    global _BUILT
    _inject_ntff_hook()
    from concourse.bass_utils import run_bass_kernel_spmd

    if _BUILT is None:
        _BUILT = _build()
    in_maps = _prep_in_maps(inputs)
    r = run_bass_kernel_spmd(_BUILT, in_maps, core_ids=list(range(NCORES)),
                             trace=trace, tmpdir=tmpdir)
    out = np.empty((B, N, D), np.float32)
    for c in range(NCORES):
        b, t0 = c // 4, (c % 4) * Q
        out[b, t0 : t0 + Q] = r.results[c]["out"]
    return out, r


def kernel(**inputs) -> np.ndarray:
    out, _ = _run(inputs, trace=False)
    return out


# revision 33
# speedup vs baseline: 1.1343x; 1.0830x over previous
"""Banded multi-headed attention (nn_BandedMultiheadedAttention) on 8 Trainium2 NeuronCores.

Sharding: data-parallel over (batch, sequence-chunk): core c handles batch c//4,
query positions [256*(c%4), 256*(c%4)+256). Band halo (max (KC-1)*dil = 248) is
loaded per-core with zero padding (projection of zero rows reproduces the
reference's bias padding exactly).

All-fp16 pipeline (fp32 PSUM accumulation everywhere, fp32 softmax). All DRAM
input layouts are host-packed so every SBUF partition's data is one contiguous
run (DMA is descriptor-rate-bound otherwise).

  1. Q/K projections (fp16): qT_s/kT_s [dh=128, pos].
  2. Banded scores per subhead: dense scores [q, m] -> deinterleaved fp16 DRAM
     plane (per (c, s) slice), shear-gather back as band [q, 32], PE-transpose
     to bandT [32, q].
  3. Pos_Sampling (Sk) matmul + Sb -> score2 [q, (head, 32)] fp32, exp, per-head
     row-sum, reciprocal, normalize -> W [q, (head, 32)] fp16.
  4. W scatter -> zeroed DRAM plane per head in [q, m] layout (contiguous runs),
     bulk readback per q-chunk, PE-transpose 128-col chunks to W^T [m, q];
     (chunk, q-half) combinations that cannot hold band data are skipped via
     partial-width PSUM accumulation in PV.
  5. V projection (fp16, heads packed in N) -> v_h [m, dh] tiles, interleaved
     with the band/softmax phase to keep PE busy during DRAM round trips.
  6. PV: attnT_h [dh, q] = v_h^T @ W^T + Vb (softmax rows sum to 1).
  7. Collapse: out [q, 640] = sum_h attnT_h^T @ CkT_h + Cb; half 0 interleaved
     with PV per head, half 1 as tail.
"""

import contextlib
import ctypes
import sys
import types

import numpy as np

# ---------------------------------------------------------------- constants
B, N, D = 2, 1024, 640
DH, KC, SUBHEADS, HEADS = 128, 32, 5, 14
Q = 256                      # query positions per core
NCORES = 8
HALO = 124                   # (KC-1)*max_dil // 2
KV = 512                     # kv halo positions per core ([t0-124, t0+388))
KVX = 1024                   # zero-extended vT columns

DIL_S = [1, 1, 2, 4, 8]
SUPER = [5, 5, 2, 1, 1]
DIL_H = [1] * 10 + [2, 2, 4, 8]
PL_S = [(KC - 1) * d // 2 for d in DIL_S]          # [15,15,31,62,124]
OFF_S = [HALO - p for p in PL_S]                   # kT col of m=0 per subhead
PL_H = [(KC - 1) * d // 2 for d in DIL_H]
OFF_H = [HALO - p for p in PL_H]

M_S = [288, 288, 320, 384, 512]                    # scores plane width per subhead
SOFF = [0, 288, 576, 896, 1280]
SLD = 1792                                         # scores plane row stride

# W plane width per head: md = M_H/dil must be a multiple of 128 so that each
# 128-col chunk of the deinterleaved plane lies in a single residue class
# (matmul stationary APs must be 2D, so V tiles need single-stride rows).
M_H = [384] * 10 + [512, 512, 512, 1024]
WOFF = [0]
for _m in M_H[:-1]:
    WOFF.append(WOFF[-1] + _m)
WLD = WOFF[-1] + M_H[-1]                           # 5632
MC_H = [m // 128 for m in M_H]

# V-projection head packs (same dilation within a pack)
PACKS = [[0, 1, 2, 3], [4, 5, 6, 7], [8, 9], [10, 11], [12], [13]]
PACK_OF_H = {h: (p, g.index(h)) for p, g in enumerate(PACKS) for h in g}
PACK_OFF = [OFF_H[g[0]] for g in PACKS]
PACK_MC = [MC_H[g[0]] for g in PACKS]

HJ = HEADS * KC  # 448


def _chunk_rows(h_or_p, mc, head=True):
    """Row (t, rho) segments of 128-col W-plane chunk mc: returns list of
    (row_in_chunk, rho, t0, seg_len); positions are p = OFF + dil*t + rho."""
    dil = DIL_H[h_or_p] if head else DIL_H[PACKS[h_or_p][0]]
    M = M_H[h_or_p] if head else M_H[PACKS[h_or_p][0]]
    md = M // dil
    segs = []
    r = 0
    while r < 128:
        col0 = mc * 128 + r
        rho, t0 = col0 // md, col0 % md
        seg = min(128 - r, md - t0)
        segs.append((r, rho, t0, seg))
        r += seg
    return segs


def _live_cs(h, mc):
    """Which q-halves c have any band data in W-plane chunk (h, mc)."""
    dil = DIL_H[h]
    lives = []
    for c in range(2):
        lo, hi = c * 128, c * 128 + 127 + (KC - 1) * dil
        ok = False
        for _, rho, t0, seg in _chunk_rows(h, mc):
            p0, p1 = dil * t0 + rho, dil * (t0 + seg - 1) + rho
            if p0 <= hi and p1 >= lo:
                ok = True
        if ok:
            lives.append(c)
    return lives


_BUILT = None


def _inject_ntff_hook():
    """bass_utils reads antenv.axon_hooks for NTFF profiling; the module is
    absent in this image. Recreate the ctypes glue (mirrors trn_boot.py)."""
    try:
        import antenv.axon_hooks  # noqa: F401
        return
    except ImportError:
        pass

    def _make(so_path):
        try:
            lib = ctypes.CDLL(so_path)
        except OSError:
            return None
        if not hasattr(lib, "axon_start_nrt_profile"):
            return None
        lib.axon_start_nrt_profile.argtypes = [ctypes.POINTER(ctypes.c_int64), ctypes.c_size_t]
        lib.axon_start_nrt_profile.restype = ctypes.c_int64
        lib.axon_stop_nrt_profile.argtypes = [ctypes.c_char_p]
        lib.axon_stop_nrt_profile.restype = ctypes.c_int64

        @contextlib.contextmanager
        def _hook(output_dir, device_ids):
            import jax
            jax.devices()
            if device_ids:
                ids = (ctypes.c_int64 * len(device_ids))(*device_ids)
                rc = lib.axon_start_nrt_profile(ids, len(device_ids))
            else:
                rc = lib.axon_start_nrt_profile(None, 0)
            if rc != 0:
                raise RuntimeError(f"axon_start_nrt_profile rc={rc}")
            try:
                yield
            finally:
                n = lib.axon_stop_nrt_profile(str(output_dir).encode())
                print(f"ntff profile: {n} file(s) -> {output_dir}", file=sys.stderr)

        return _hook

    hook = _make("/opt/axon/libaxon_pjrt.so")
    mod = types.ModuleType("antenv.axon_hooks")
    mod.get_axon_ntff_profile_hook = lambda: hook
    mod.set_axon_ntff_profile_hook = lambda h: None
    sys.modules["antenv.axon_hooks"] = mod


def _build():
    """Build the (single) SPMD Bass program. Returns finalized nc."""
    import concourse.bass as bass
    import concourse.tile as tile
    from concourse import bacc, mybir
    from concourse.masks import make_identity
    from concourse.tile import add_dep_helper

    f32 = mybir.dt.float32
    f16 = mybir.dt.float16
    AP = bass.AP

    nc = bacc.Bacc("TRN2", target_bir_lowering=False, debug=False, num_devices=NCORES)

    # ---------------- external IO (all fp16 except fp32 biases / output)
    # every input is host-packed [128, free] partition-major
    qT_d = nc.dram_tensor("qT", [DH, SUBHEADS * Q], f16, kind="ExternalInput")
    kT_d = nc.dram_tensor("kT", [DH, SUBHEADS * KV], f16, kind="ExternalInput")
    vT_d = nc.dram_tensor("vT", [DH, SUBHEADS * KV], f16, kind="ExternalInput")
    QkT_d = nc.dram_tensor("QkT", [DH, SUBHEADS * SUBHEADS * DH], f16, kind="ExternalInput")
    KkT_d = nc.dram_tensor("KkT", [DH, SUBHEADS * SUBHEADS * DH], f16, kind="ExternalInput")
    # V weights in three pack groups: heads 0-3, 4-7, 8-13
    VG = [512, 512, 768]
    VGP = [[0], [1], [2, 3, 4, 5]]  # packs per group
    Vg_d = [nc.dram_tensor(f"Vg{i}", [DH, SUBHEADS * w], f16, kind="ExternalInput")
            for i, w in enumerate(VG)]
    SkT_d = nc.dram_tensor("SkT", [KC, HJ], f16, kind="ExternalInput")
    Sb_d = nc.dram_tensor("Sb", [1, HJ], f32, kind="ExternalInput")
    bias3_d = nc.dram_tensor("bias3", [DH, 2 * SUBHEADS + HEADS], f32,
                             kind="ExternalInput")
    CkT_d = nc.dram_tensor("CkT", [DH, HEADS * D], f16, kind="ExternalInput")
    Cb_d = nc.dram_tensor("Cb", [1, D], f32, kind="ExternalInput")
    out_d = nc.dram_tensor("out", [Q, D], f32, kind="ExternalOutput")

    # ---------------- internal DRAM scratch, split per q-chunk so the tile
    # framework's DRAM dependency tracking stays per-chunk.
    splane = [nc.dram_tensor(f"splane{c}", [128, SLD], f16, kind="Internal")
              for c in range(2)]
    wplane = [nc.dram_tensor(f"wplane{c}", [128, WLD], f16, kind="Internal")
              for c in range(2)]

    with tile.TileContext(nc) as tc, contextlib.ExitStack() as ctx:
        consts = ctx.enter_context(tc.tile_pool(name="consts", bufs=1))
        acts = ctx.enter_context(tc.tile_pool(name="acts", bufs=1))
        work = ctx.enter_context(tc.tile_pool(name="work", bufs=4))
        wftp = ctx.enter_context(tc.tile_pool(name="wft", bufs=6))
        actp = ctx.enter_context(tc.tile_pool(name="actp", bufs=2))
        ps_mm = ctx.enter_context(tc.tile_pool(name="ps_mm", bufs=2, space="PSUM"))
        ps_sm = ctx.enter_context(tc.tile_pool(name="ps_sm", bufs=2, space="PSUM"))
        ps_at = ctx.enter_context(tc.tile_pool(name="ps_at", bufs=2, space="PSUM"))
        ps_co = ctx.enter_context(tc.tile_pool(name="ps_co", bufs=2, space="PSUM"))

        eng2 = [nc.sync, nc.scalar]
        cpy2 = [nc.scalar, nc.vector]

        # ---------------- critical inputs first: qT + QkT (then kT + KkT),
        # halves split across the two HWDGE engines; big contiguous runs.
        qTr = acts.tile([DH, SUBHEADS, Q], f16)
        kTr = acts.tile([DH, SUBHEADS, KV], f16)
        QkTr = consts.tile([DH, SUBHEADS * SUBHEADS, DH], f16)
        KkTr = consts.tile([DH, SUBHEADS * SUBHEADS, DH], f16)

        def pieces(eng, dst, src_d, width, npc):
            ds = []
            for i in range(npc):
                a, b = width * i // npc, width * (i + 1) // npc
                ds.append(eng.dma_start(
                    out=dst[:, a:b], in_=AP(src_d, a, [[width, DH], [1, b - a]])))
            return ds

        qTrf = qTr.rearrange("p a b -> p (a b)")
        kTrf = kTr.rearrange("p a b -> p (a b)")
        QkTrf = QkTr.rearrange("p a b -> p (a b)")
        KkTrf = KkTr.rearrange("p a b -> p (a b)")
        g1 = pieces(nc.sync, qTrf, qT_d, SUBHEADS * Q, 2)
        g2 = pieces(nc.scalar, QkTrf, QkT_d, SUBHEADS * SUBHEADS * DH, 3)
        bias3 = consts.tile([DH, 2 * SUBHEADS + HEADS], f32)
        nc.sync.dma_start(out=bias3, in_=bias3_d.ap())
        g3 = pieces(nc.sync, kTrf, kT_d, SUBHEADS * KV, 2)
        g4 = pieces(nc.scalar, KkTrf, KkT_d, SUBHEADS * SUBHEADS * DH, 3)
        gates = [g1[-1], g2[-1], g3[-1], g4[-1]]
        QbT = bias3[:, 0:SUBHEADS]
        KbT = bias3[:, SUBHEADS : 2 * SUBHEADS]
        VbT = bias3[:, 2 * SUBHEADS :]

        # ---------------- bulk loads on SWDGE (Pool engine)
        def gated(d):
            for g in gates:
                add_dep_helper(d.ins, g.ins, sync=True,
                               reason="defer bulk DMA until critical inputs loaded")
            return d

        vT = acts.tile([DH, SUBHEADS, KVX], f16)
        nc.vector.memset(vT[:, :, KV:], 0.0)
        gated(nc.gpsimd.dma_start(
            out=AP(vT.tensor, vT.offset,
                   [[SUBHEADS * KVX, DH], [KVX, SUBHEADS], [1, KV]]),
            in_=AP(vT_d, 0, [[SUBHEADS * KV, DH], [KV, SUBHEADS], [1, KV]])))
        Vgt = [consts.tile([DH, SUBHEADS, w], f16, name=f"Vg{i}")
               for i, w in enumerate(VG)]
        for i in range(3):
            gated(nc.gpsimd.dma_start(out=Vgt[i].rearrange("p a b -> p (a b)"),
                                      in_=Vg_d[i].ap()))
        # per-pack views into the groups
        Vp = []
        for i, ps in enumerate(VGP):
            off = 0
            for p in ps:
                npk = len(PACKS[p]) * DH
                Vp.append(Vgt[i][:, :, off : off + npk])
                off += npk
        SkT = consts.tile([KC, HJ], f16)
        nc.gpsimd.dma_start(out=SkT, in_=SkT_d.ap())
        Sb = consts.tile([DH, HJ], f32)
        nc.gpsimd.dma_start(out=Sb, in_=AP(Sb_d, 0, [[0, DH], [1, HJ]]))

        # zero the W planes (one fat DMA per plane; small zero source repeated)
        zrow = work.tile([DH, WLD // 5], f16, name="zrow", tag="zr", bufs=1)
        nc.vector.memset(zrow, 0.0)
        for c in range(2):
            gated(nc.gpsimd.dma_start(
                out=AP(wplane[c], 0, [[WLD, 128], [1, WLD]]),
                in_=AP(zrow.tensor, zrow.offset,
                       [[WLD // 5, DH], [0, 5], [1, WLD // 5]])))

        CkT = consts.tile([DH, HEADS, D], f16)   # f-chunk h on partitions' free dim
        gated(nc.gpsimd.dma_start(out=CkT.rearrange("p a b -> p (a b)"),
                                  in_=CkT_d.ap()))
        Cb = consts.tile([DH, D], f32)
        gated(nc.gpsimd.dma_start(out=Cb, in_=AP(Cb_d, 0, [[0, DH], [1, D]])))

        ident = consts.tile([DH, DH], f32)
        make_identity(nc, ident)
        identh = consts.tile([DH, DH], f16)
        nc.vector.tensor_copy(identh, ident)

        # ---------------- Q/K projections (fp16 operands, fp32 PSUM)
        qTs, kTs = [], []
        for s in range(SUBHEADS):
            pq = ps_mm.tile([DH, Q], f32, name=f"pq{s}", tag="mm")
            for dc in range(SUBHEADS):
                nc.tensor.matmul(pq, QkTr[:, s * SUBHEADS + dc, :], qTr[:, dc, :],
                                 start=(dc == 0), stop=(dc == SUBHEADS - 1))
            t = acts.tile([DH, Q], f16, name=f"qTs{s}")
            nc.scalar.activation(t, pq, mybir.ActivationFunctionType.Identity,
                                 bias=QbT[:, s : s + 1], scale=1.0)
            qTs.append(t)

            # K projection only over the kT window this subhead's scores read
            ms = M_S[s]
            pk = ps_mm.tile([DH, ms], f32, name=f"pk{s}", tag="mm")
            for dc in range(SUBHEADS):
                nc.tensor.matmul(pk,
                                 KkTr[:, s * SUBHEADS + dc, :],
                                 kTr[:, dc, OFF_S[s] : OFF_S[s] + ms],
                                 start=(dc == 0), stop=(dc == SUBHEADS - 1))
            t = acts.tile([DH, ms], f16, name=f"kTs{s}")
            nc.vector.tensor_add(
                t, pk, AP(bias3.tensor,
                          bias3.offset + SUBHEADS + s,
                          [[2 * SUBHEADS + HEADS, DH], [0, ms]]))
            kTs.append(t)

        # ---------------- banded scores -> deinterleaved DRAM planes
        # per (c, s) slice DMAs so the band gathers can start per subhead
        bands = {}  # (c, s) -> [128, KC] f16 view
        for c in range(2):
            ssb = work.tile([128, SLD], f16, name=f"ssb{c}", tag="ssb", bufs=2)
            for s in range(SUBHEADS):
                dil, ms = DIL_S[s], M_S[s]
                pscore = ps_mm.tile([128, ms], f32, name=f"psc{s}{c}", tag="mm")
                nc.tensor.matmul(pscore, qTs[s][:, c * 128 : c * 128 + 128],
                                 kTs[s], start=True, stop=True)
                if dil == 1:
                    psrc = pscore
                    dst = ssb[:, SOFF[s] : SOFF[s] + ms]
                else:
                    # deinterleave m -> (m%dil, m//dil) during PSUM->SBUF copy
                    psrc = AP(pscore.tensor, pscore.offset,
                              [[ms, 128], [1, dil], [dil, ms // dil]])
                    dst = AP(ssb.tensor, ssb.offset + SOFF[s],
                             [[SLD, 128], [ms // dil, dil], [1, ms // dil]])
                if c == 0:
                    nc.vector.tensor_copy(dst, psrc)
                else:
                    nc.scalar.copy(dst, psrc)
            nc.sync.dma_start(
                out=AP(splane[c], 0, [[SLD, 128], [1, SLD]]), in_=ssb)
            band01 = work.tile([128, 2, KC], f16, name=f"band01_{c}", tag="band01",
                               bufs=2)
            nc.sync.dma_start(
                out=band01,
                in_=AP(splane[c], c * 128, [[SLD + 1, 128], [SOFF[1], 2], [1, KC]]))
            bands[(c, 0)] = band01[:, 0, :]
            bands[(c, 1)] = band01[:, 1, :]
            for s in range(2, SUBHEADS):
                dil, ms = DIL_S[s], M_S[s]
                band = work.tile([128, KC], f16, name=f"band{c}{s}", tag="band",
                                 bufs=6)
                nc.sync.dma_start(
                    out=band,
                    in_=AP(splane[c], SOFF[s] + (c * 128) // dil,
                           [[SLD + ms // dil, dil], [dil * SLD + 1, 128 // dil],
                            [1, KC]]))
                bands[(c, s)] = band

        # ---------------- V projection tiles (fp16), interleaved with the
        # band->Sk->softmax phase so the PE stays busy during DRAM round trips.
        vtiles = {}  # (pack, mc) -> [128, len(g)*128] f16; rows in deint m-order

        def vproj_packs(plist):
            for p in plist:
                g = PACKS[p]
                npk = len(g) * DH
                dil = DIL_H[g[0]]
                for mc in range(PACK_MC[p]):
                    pv = ps_mm.tile([128, npk], f32, name=f"pv{p}{mc}", tag="mm")
                    segs = _chunk_rows(p, mc, head=False)
                    for dc in range(SUBHEADS):
                        base = vT.offset + dc * KVX
                        if len(segs) == 1:
                            _, rho, t0, _ = segs[0]
                            lhsT = AP(vT.tensor, base + PACK_OFF[p] + dil * t0 + rho,
                                      [[SUBHEADS * KVX, DH], [dil, 128]])
                        else:
                            # md=64: two residues per chunk; rows = (rho, rho+1) x t
                            _, rho, t0, seg = segs[0]
                            assert seg == 64 and t0 == 0
                            lhsT = AP(vT.tensor, base + PACK_OFF[p] + rho,
                                      [[SUBHEADS * KVX, DH], [1, 2], [dil, 64]])
                        nc.tensor.matmul(pv, lhsT, Vp[p][:, dc, :],
                                         start=(dc == 0), stop=(dc == SUBHEADS - 1))
                    t = acts.tile([128, npk], f16, name=f"v{p}_{mc}")
                    if (p + mc) % 2 == 0:
                        nc.vector.tensor_copy(t, pv)
                    else:
                        nc.scalar.copy(t, pv)
                    vtiles[(p, mc)] = t

        # ---------------- band -> Sk -> softmax -> scatter, per chunk
        def band_phase(c):
            bandTs = []
            for s in range(SUBHEADS):
                pbt = ps_sm.tile([KC, 128], f16, name="pbt", tag="sm")
                nc.tensor.transpose(pbt, bands[(c, s)], identh)
                bt = work.tile([KC, 128], f16, name="bt", tag="bt", bufs=6)
                nc.scalar.copy(bt, pbt)
                bandTs.append(bt)

            e = work.tile([128, HJ], f32, name="e", tag="e", bufs=2)
            hlo = 0
            for s in range(SUBHEADS):
                ncols = SUPER[s] * KC
                psk = ps_sm.tile([128, ncols], f32, name="psk", tag="sm")
                nc.tensor.matmul(psk, bandTs[s], SkT[:, hlo : hlo + ncols],
                                 start=True, stop=True)
                nc.vector.tensor_add(e[:, hlo : hlo + ncols], psk,
                                     Sb[:, hlo : hlo + ncols])
                hlo += ncols
            nc.scalar.activation(e, e, mybir.ActivationFunctionType.Exp)
            z = work.tile([128, HEADS], f32, name="z", tag="z", bufs=4)
            nc.vector.reduce_sum(z, e.rearrange("p (h k) -> p h k", k=KC),
                                 axis=mybir.AxisListType.X)
            rz = work.tile([128, HEADS], f32, name="rz", tag="z", bufs=4)
            nc.vector.reciprocal(rz, z)
            w = work.tile([128, HJ], f16, name="w", tag="w", bufs=2)
            nc.vector.tensor_mul(
                w.rearrange("p (h k) -> p h k", k=KC),
                e.rearrange("p (h k) -> p h k", k=KC),
                AP(rz.tensor, rz.offset, [[HEADS, 128], [1, HEADS], [0, KC]]),
            )

            # scatter W into the zeroed plane ([q, m] layout, contiguous runs)
            # dil=1 heads 0..9 merged into one DMA
            nc.sync.dma_start(
                out=AP(wplane[c], c * 128, [[WLD + 1, 128], [384, 10], [1, KC]]),
                in_=AP(w.tensor, w.offset, [[HJ, 128], [KC, 10], [1, KC]]),
            )
            for h in range(10, HEADS):
                dil, mh = DIL_H[h], M_H[h]
                base = WOFF[h] + (c * 128) // dil
                nc.sync.dma_start(
                    out=AP(wplane[c], base,
                           [[WLD + mh // dil, dil], [dil * WLD + 1, 128 // dil],
                            [1, KC]]),
                    in_=AP(w.tensor, w.offset + h * KC, [[HJ, 128], [1, KC]]),
                )
            # bulk readback of this chunk's W rows (waits on the scatters),
            # split in head-consumption order so the first W^T transposes
            # start after the first piece lands
            t = acts.tile([128, WLD], f16, name=f"wpl{c}")
            for i in range(2):
                a, b = WLD * i // 2, WLD * (i + 1) // 2
                nc.sync.dma_start(
                    out=t[:, a:b], in_=AP(wplane[c], a, [[WLD, 128], [1, b - a]]))
            return t

        # interleave: bands c0, Vproj pack1, bands c1, Vproj packs 2-5
        wpl = [None, None]
        vproj_packs([0])
        wpl[0] = band_phase(0)
        vproj_packs([1])
        wpl[1] = band_phase(1)
        vproj_packs([2, 3, 4, 5])

        # ---------------- W^T via PE transposes + PV + collapse, one q-chunk
        # stream at a time: the c0 stream depends only on wplane[0]'s readback,
        # so it runs (and writes its output) while the c1 scatter/readback
        # chain completes in the background.
        atiles = {}  # (h, c) -> [DH, 128] f16
        cpy3 = [nc.vector, nc.scalar]
        ncp = 0
        outsb = [work.tile([128, D], f32, name=f"osb{c}", tag="osb", bufs=2)
                 for c in range(2)]

        def out_dma(cc, half):
            for i in range(2):
                a = half * 320 + i * 160
                eng2[(cc + i) % 2].dma_start(
                    out=AP(out_d, cc * 128 * D + a, [[D, 128], [1, 160]]),
                    in_=outsb[cc][:, a : a + 160])

        for c in range(2):
            pcs = {}

            def collapse_c(h, half, start, stop):
                if half not in pcs:
                    pcs[half] = ps_co.tile([128, 320], f32, name=f"pc{c}{half}",
                                           tag="co")
                nc.tensor.matmul(pcs[half], atiles[(h, c)],
                                 CkT[:, h, half * 320 : half * 320 + 320],
                                 start=start, stop=stop, skip_group_check=True)

            for h in range(HEADS):
                p, hh = PACK_OF_H[h]
                pat = ps_at.tile([DH, 128], f32, name=f"pat{h}{c}", tag="at")
                mcs = [mc for mc in range(MC_H[h]) if c in _live_cs(h, mc)]
                # batches of <=4 chunks: 4 PE transposes feed ONE PSUM->SBUF
                # copy (copies are fixed-overhead dominated)
                batches = [mcs[i : i + 4] for i in range(0, len(mcs), 4)]
                pos = {}
                for batch in batches:
                    nb = len(batch)
                    ptp = ps_sm.tile([128, nb * 128], f16, name="ptp", tag="sm")
                    for i, mc in enumerate(batch):
                        nc.tensor.transpose(
                            ptp[:, i * 128 : i * 128 + 128],
                            wpl[c][:, WOFF[h] + mc * 128 : WOFF[h] + mc * 128 + 128],
                            identh)
                    wt = wftp.tile([128, nb * 128], f16, name="wft", tag="wft")
                    eng = cpy3[ncp % 2]
                    if eng is nc.scalar:
                        eng.copy(wt, ptp)
                    else:
                        eng.tensor_copy(wt, ptp)
                    ncp += 1
                    for i, mc in enumerate(batch):
                        pos[mc] = (wt, i)
                for i, mc in enumerate(mcs):
                    wt, j = pos[mc]
                    nc.tensor.matmul(pat,
                                     vtiles[(p, mc)][:, hh * DH : hh * DH + DH],
                                     wt[:, j * 128 : j * 128 + 128],
                                     start=(i == 0), stop=(i == len(mcs) - 1),
                                     skip_group_check=True)
                at = actp.tile([DH, 128], f16, name=f"at{h}{c}", tag="at", bufs=28)
                nc.vector.tensor_add(
                    at, pat,
                    AP(bias3.tensor, bias3.offset + 2 * SUBHEADS + h,
                       [[2 * SUBHEADS + HEADS, DH], [0, 128]]))
                atiles[(h, c)] = at
                if h > 0:
                    collapse_c(h - 1, 0, start=(h == 1), stop=False)
            collapse_c(HEADS - 1, 0, start=False, stop=True)
            nc.vector.tensor_add(outsb[c][:, 0:320], pcs[0], Cb[:, 0:320])
            out_dma(c, 0)
            for h in range(HEADS):
                collapse_c(h, 1, start=(h == 0), stop=(h == HEADS - 1))
            nc.vector.tensor_add(outsb[c][:, 320:640], pcs[1], Cb[:, 320:640])
            out_dma(c, 1)

    nc.finalize()
    return nc


def _pack_rows(x, nchunk):
    """[nchunk*128, F] -> [128, nchunk*F] partition-major contiguous."""
    F = x.shape[1]
    return np.ascontiguousarray(
        x.reshape(nchunk, DH, F).transpose(1, 0, 2).reshape(DH, nchunk * F))


def _prep_in_maps(inputs):
    h16 = np.float16
    query = np.asarray(inputs["query"], np.float32)
    key = np.asarray(inputs["key"], np.float32)
    value = np.asarray(inputs["value"], np.float32)
    Qk = np.asarray(inputs["Qk"], np.float32)
    Qb = np.asarray(inputs["Qb"], np.float32)
    Kk = np.asarray(inputs["Kk"], np.float32)
    Kb = np.asarray(inputs["Kb"], np.float32)
    Vk = np.asarray(inputs["Vk"], np.float32)
    Vb = np.asarray(inputs["Vb"], np.float32)
    Sk = np.asarray(inputs["Sk"], np.float32)
    Sb = np.asarray(inputs["Sb"], np.float32)
    Ck = np.asarray(inputs["Ck"], np.float32)
    Cb = np.asarray(inputs["Cb"], np.float32)

    # QkT packed: [128, (s*5+dc)*128] with [d2, ., o] = Qk[s, o, dc*128+d2]
    def pack_w(Wk):  # [5, 128, 640] -> [128 d2, 25*128]
        WkT = Wk.transpose(0, 2, 1).reshape(SUBHEADS, SUBHEADS, DH, DH)  # s,dc,d2,o
        return np.ascontiguousarray(
            WkT.transpose(2, 0, 1, 3).reshape(DH, SUBHEADS * SUBHEADS * DH)).astype(h16)

    QkTp = pack_w(Qk)
    KkTp = pack_w(Kk)
    VkT = Vk.transpose(0, 2, 1)                                    # [14, 640, 128]
    VGH = [[0, 1, 2, 3], [4, 5, 6, 7], [8, 9, 10, 11, 12, 13]]     # heads per group
    Vgp = [_pack_rows(np.concatenate([VkT[h] for h in g], axis=1), SUBHEADS).astype(h16)
           for g in VGH]
    SkT = np.ascontiguousarray(Sk.transpose(2, 0, 1).reshape(KC, HJ)).astype(h16)
    Sbr = np.ascontiguousarray(Sb.reshape(1, HJ))
    bias3 = np.ascontiguousarray(
        np.concatenate([Qb.T, Kb.T, Vb.T], axis=1))                # [128, 24]
    CkTp = _pack_rows(np.ascontiguousarray(Ck.T), HEADS).astype(h16)  # [128, 14*640]
    Cbr = np.ascontiguousarray(Cb.reshape(1, D))

    in_maps = []
    for c in range(NCORES):
        b, t0 = c // 4, (c % 4) * Q
        kpad = np.zeros((KV, D), np.float32)
        vpad = np.zeros((KV, D), np.float32)
        lo, hi = max(0, t0 - HALO), min(N, t0 + Q + 132)
        kpad[lo - (t0 - HALO) : hi - (t0 - HALO)] = key[b, lo:hi]
        vpad[lo - (t0 - HALO) : hi - (t0 - HALO)] = value[b, lo:hi]
        m = {
            "qT": _pack_rows(query[b, t0 : t0 + Q].T, SUBHEADS).astype(h16),
            "kT": _pack_rows(kpad.T, SUBHEADS).astype(h16),
            "vT": _pack_rows(vpad.T, SUBHEADS).astype(h16),
            "QkT": QkTp, "KkT": KkTp,
            "SkT": SkT, "Sb": Sbr, "bias3": bias3,
            "CkT": CkTp, "Cb": Cbr,
        }
        for i in range(3):
            m[f"Vg{i}"] = Vgp[i]
        in_maps.append(m)
    return in_maps


def _run(inputs, trace=False, tmpdir=None):
    global _BUILT
    _inject_ntff_hook()
    from concourse.bass_utils import run_bass_kernel_spmd

    if _BUILT is None:
        _BUILT = _build()
    in_maps = _prep_in_maps(inputs)
    r = run_bass_kernel_spmd(_BUILT, in_maps, core_ids=list(range(NCORES)),
                             trace=trace, tmpdir=tmpdir)
    out = np.empty((B, N, D), np.float32)
    for c in range(NCORES):
        b, t0 = c // 4, (c % 4) * Q
        out[b, t0 : t0 + Q] = r.results[c]["out"]
    return out, r


def kernel(**inputs) -> np.ndarray:
    out, _ = _run(inputs, trace=False)
    return out
